# revision 9
# baseline (speedup 1.0000x reference)
"""Trainium2 Bass kernel for nn_ContrastiveDistortion (symmetric pairwise-KL InfoNCE loss).

Math: with IS_SYMMETRIC=True the logdet terms cancel and
  logits_sym[a,b] = D/2 - U[a,b]/4
With w = sigma^2 + mu^2, inv = 1/sigma^2, m2i = -2*mu*inv, niv = mu*inv,
mu2 = -2*mu:
  U[a,b] = <inv_a, w_b> + <w_a, inv_b> + <m2i_a, mu_b> + <mu2_a, niv_b> + c_b + c_a
where c_x = sum_d (mu^2*inv)_x. The row term c_a cancels in log-softmax, and
c_b is computed as sum_d (w*inv)_b = c_b + D (the uniform +D also cancels), so
only FIVE K=128 matmul chunks per output tile are needed (vs 7 naive).

Each of the 8 cores gets the full [128,4096] feature-major mu/sigma,
column-ROTATED by 512*k so the program is SPMD-identical: the core's own
512-row block is always local columns 0..511 (diagonal masked there) and the
positive pairs are local columns 2048..2559.

Pipeline over eight 512-column tiles: DMA prefetch (2 ahead) -> plane prep
(1 ahead, spread over Act/DVE/Pool) -> 20 matmuls/tile -> online-softmax
consumers. PSUM holds four [128,1024] accumulators (one per 128-row m-chunk,
covering a PAIR of column tiles = all 16KB of PSUM), so the min-reduce (Pool)
and exp+accumulate (Act) each run once per 1024 columns. 16 per-(m,pair)
partial (min, sum-exp) pairs are combined with a small logsumexp tree at the
end; per-core partial row-loss sums are reduced on host.
"""

import sys
from contextlib import ExitStack

import numpy as np

sys.path.insert(0, "/opt/trn_rl_repo")

import concourse.bass as bass
import concourse.bacc as bacc_mod
import concourse.mybir as mybir
from concourse.bass_utils import run_bass_kernel_spmd
from concourse.tile import TileContext

F32 = mybir.dt.float32
F32R = mybir.dt.float32r
I32 = mybir.dt.int32
AF = mybir.ActivationFunctionType
ALU = mybir.AluOpType
AX = mybir.AxisListType

P = 128          # partitions / feature dim D
NB = 4096        # N = 2B rows
NC = 8           # cores
RB = NB // NC    # 512 rows per core
NM = RB // P     # 4 m-chunks of 128 rows
CT = 512         # column tile
NCT = NB // CT   # 8 column tiles
NPAIR = NCT // 2  # 4 column-tile pairs (one PSUM accumulator lifetime each)
NPART = NM * NPAIR  # 16 (m, pair) softmax partials
TEMPERATURE = 0.1
WEIGHT = 5.0
SCL = 1.0 / (4.0 * TEMPERATURE)  # 2.5: logit = -SCL*U + const_row
BIG = 1e30


def _build_nc():
    nc = bacc_mod.Bacc(None, target_bir_lowering=False, name="contrastive_distortion")
    muT_d = nc.declare_dram_parameter("muT", [P, NB], F32R, isOutput=False)
    sgT_d = nc.declare_dram_parameter("sigmaT", [P, NB], F32R, isOutput=False)
    out_d = nc.declare_dram_parameter("out", [P, NM], F32, isOutput=True)

    with TileContext(nc) as tc, ExitStack() as ctx:
        per = ctx.enter_context(tc.tile_pool(name="per", bufs=1))
        io = ctx.enter_context(tc.tile_pool(name="io", bufs=3))
        st = ctx.enter_context(tc.tile_pool(name="st", bufs=2))
        sc = ctx.enter_context(tc.tile_pool(name="sc", bufs=2))
        pp = ctx.enter_context(tc.tile_pool(name="pp", bufs=1, space="PSUM"))

        ones_f = per.tile([P, P], F32)
        ones = per.tile([P, P], F32R)
        ioti = per.tile([P, P], I32)
        I128 = per.tile([P, P], F32)
        pmin = per.tile([P, NPART], F32)
        biasb = per.tile([P, NPART], F32)
        esum = per.tile([P, NPART], F32)
        upos4 = per.tile([P, NM], F32)

        nc.vector.memset(ones_f, 1.0)
        nc.vector.tensor_copy(out=ones, in_=ones_f)
        # I128[p, c] = (c - p == 0): identity for diag mask / positive extract
        nc.gpsimd.iota(ioti, pattern=[[1, P]], base=0, channel_multiplier=-1)
        nc.vector.tensor_single_scalar(out=I128, in_=ioti, scalar=0,
                                       op=ALU.is_equal)

        # persistent ct0 planes: lhsT for every matmul + rhs for ct0
        mu0 = per.tile([P, CT], F32R)
        sg0 = per.tile([P, CT], F32R)
        var0 = per.tile([P, CT], F32R)
        msq0 = per.tile([P, CT], F32R)
        inv0 = per.tile([P, CT], F32R)
        w0 = per.tile([P, CT], F32R)
        m2i0 = per.tile([P, CT], F32R)
        mu20 = per.tile([P, CT], F32R)
        niv0 = per.tile([P, CT], F32R)
        wiv0 = per.tile([P, CT], F32R)

        def dma_in(ct):
            mu_c = io.tile([P, CT], F32R, name=f"mu{ct}", tag="mu")
            sg_c = io.tile([P, CT], F32R, name=f"sg{ct}", tag="sg")
            nc.sync.dma_start(out=mu_c, in_=muT_d[:, CT * ct:CT * (ct + 1)])
            nc.sync.dma_start(out=sg_c, in_=sgT_d[:, CT * ct:CT * (ct + 1)])
            return mu_c, sg_c

        nc.sync.dma_start(out=mu0, in_=muT_d[:, 0:CT])
        nc.sync.dma_start(out=sg0, in_=sgT_d[:, 0:CT])
        iot = {1: dma_in(1)}

        # ct0 planes: latency-optimized engine spread (Act/DVE/Pool parallel)
        nc.scalar.activation(out=var0, in_=sg0, func=AF.Square)
        nc.vector.tensor_mul(msq0, mu0, mu0)
        with nc.allow_low_precision("planes feed the PE which reads fp22"):
            nc.vector.reciprocal(inv0, var0)
        nc.gpsimd.tensor_add(w0, var0, msq0)
        nc.vector.scalar_tensor_tensor(out=m2i0, in0=mu0, scalar=-2.0,
                                       in1=inv0, op0=ALU.mult, op1=ALU.mult)
        nc.vector.tensor_scalar_mul(mu20, mu0, -2.0)
        nc.gpsimd.tensor_mul(niv0, mu0, inv0)
        nc.gpsimd.tensor_mul(wiv0, w0, inv0)
        planes = {0: (mu0, w0, inv0, niv0, wiv0)}

        def mk_planes(ct, mu_c, sg_c):
            # steady state: Act squares, DVE reciprocal (+pair mins), Pool
            # the TensorTensor ops (walrus rejects TensorScalar* on Pool)
            var = st.tile([P, CT], F32R, name=f"var{ct}", tag="var")
            msq = st.tile([P, CT], F32R, name=f"msq{ct}", tag="msq")
            inv = st.tile([P, CT], F32R, name=f"inv{ct}", tag="inv")
            w = st.tile([P, CT], F32R, name=f"w{ct}", tag="w")
            niv = st.tile([P, CT], F32R, name=f"niv{ct}", tag="niv")
            wiv = st.tile([P, CT], F32R, name=f"wiv{ct}", tag="wiv")
            nc.scalar.activation(out=var, in_=sg_c, func=AF.Square)
            nc.scalar.activation(out=msq, in_=mu_c, func=AF.Square)
            with nc.allow_low_precision("planes feed the PE which reads fp22"):
                nc.vector.reciprocal(inv, var)
            nc.gpsimd.tensor_add(w, var, msq)
            nc.gpsimd.tensor_mul(niv, mu_c, inv)
            nc.gpsimd.tensor_mul(wiv, w, inv)
            return (mu_c, w, inv, niv, wiv)

        psum = {}

        def mk_mms(ct):
            mu_c, w, inv, niv, wiv = planes[ct]
            pair, half = divmod(ct, 2)
            osl = slice(CT * half, CT * (half + 1))
            for m in range(NM):
                mblk = slice(P * m, P * (m + 1))
                if half == 0:
                    psum[(pair, m)] = pp.tile([P, 2 * CT], F32,
                                              name=f"u{pair}_{m}", tag=f"ps{m}")
                u = psum[(pair, m)]
                # rhs=mu first: its DMA finished long ago, keeping the
                # group-head matmul's wait set minimal
                chunks = [(m2i0[:, mblk], mu_c), (inv0[:, mblk], w),
                          (w0[:, mblk], inv), (mu20[:, mblk], niv),
                          (ones, wiv)]
                for ci, (lt, rt) in enumerate(chunks):
                    nc.tensor.matmul(u[:, osl], lhsT=lt, rhs=rt,
                                     start=(ci == 0),
                                     stop=(ci == len(chunks) - 1))

        def mk_cons(pair):
            for m in range(NM):
                u = psum[(pair, m)]
                idx = NPAIR * m + pair
                dsl = slice(P * m, P * (m + 1))  # within even-ct half
                if pair == 0:
                    # exclude the diagonal (always local cols 128m+p)
                    nc.vector.scalar_tensor_tensor(
                        out=u[:, dsl], in0=I128, scalar=BIG, in1=u[:, dsl],
                        op0=ALU.mult, op1=ALU.add)
                if pair == 2:
                    # positive logits live at local cols 2048+128m+p
                    s128 = sc.tile([P, P], F32, name=f"s{m}", tag="s128",
                                   bufs=1)
                    nc.vector.tensor_mul(s128, u[:, dsl], I128)
                    nc.vector.tensor_reduce(upos4[:, m:m + 1], s128,
                                            axis=AX.X, op=ALU.add)
                nc.vector.tensor_reduce(pmin[:, idx:idx + 1], u, axis=AX.X,
                                        op=ALU.min)
                nc.vector.tensor_scalar_mul(biasb[:, idx:idx + 1],
                                            pmin[:, idx:idx + 1], SCL)
                e = sc.tile([P, 2 * CT], F32, name=f"e{pair}_{m}", tag="e",
                            bufs=2)
                nc.scalar.activation(out=e, in_=u, func=AF.Exp,
                                     bias=biasb[:, idx:idx + 1], scale=-SCL,
                                     accum_out=esum[:, idx:idx + 1])

        for ct in range(NCT):
            if ct + 2 < NCT:
                iot[ct + 2] = dma_in(ct + 2)
            if ct + 1 < NCT:
                planes[ct + 1] = mk_planes(ct + 1, *iot.pop(ct + 1))
            mk_mms(ct)
            if ct % 2 == 1:
                mk_cons(ct // 2)

        # tail: per-row LSE over the 4 per-pair partials; row_loss = LSE + SCL*upos
        ln16 = per.tile([P, NPART], F32)
        L16 = per.tile([P, NPART], F32)
        M4 = per.tile([P, NM], F32)
        negM4 = per.tile([P, NM], F32)
        e16 = per.tile([P, NPART], F32)
        S4 = per.tile([P, NM], F32)
        lnS4 = per.tile([P, NM], F32)
        LSE4 = per.tile([P, NM], F32)
        rl4 = per.tile([P, NM], F32)
        nc.scalar.activation(out=ln16, in_=esum, func=AF.Ln)
        nc.vector.scalar_tensor_tensor(out=L16, in0=pmin, scalar=-SCL,
                                       in1=ln16, op0=ALU.mult, op1=ALU.add)
        for m in range(NM):
            msl = slice(NPAIR * m, NPAIR * (m + 1))
            nc.vector.tensor_reduce(M4[:, m:m + 1], L16[:, msl], axis=AX.X,
                                    op=ALU.max)
        nc.vector.tensor_scalar_mul(negM4, M4, -1.0)
        for m in range(NM):
            msl = slice(NPAIR * m, NPAIR * (m + 1))
            nc.scalar.activation(out=e16[:, msl], in_=L16[:, msl], func=AF.Exp,
                                 bias=negM4[:, m:m + 1], scale=1.0)
            nc.vector.tensor_reduce(S4[:, m:m + 1], e16[:, msl], axis=AX.X,
                                    op=ALU.add)
        nc.scalar.activation(out=lnS4, in_=S4, func=AF.Ln)
        nc.vector.tensor_add(LSE4, M4, lnS4)
        nc.vector.scalar_tensor_tensor(out=rl4, in0=upos4, scalar=SCL,
                                       in1=LSE4, op0=ALU.mult, op1=ALU.add)
        nc.sync.dma_start(out=out_d[:, :], in_=rl4)

    return nc


_NC_CACHE = None


def _get_nc():
    global _NC_CACHE
    if _NC_CACHE is None:
        nc = _build_nc()
        nc.finalize()  # runs Bacc.compile(): wait legalization for TRN2
        _NC_CACHE = nc
    return _NC_CACHE


def run_sharded(mu_x, sigma_x, mu_p, sigma_p, trace=False):
    mus = np.concatenate([np.asarray(mu_x, np.float32),
                          np.asarray(mu_p, np.float32)], 0)
    sigmas = np.concatenate([np.asarray(sigma_x, np.float32),
                             np.asarray(sigma_p, np.float32)], 0)
    muT = np.ascontiguousarray(mus.T)
    sgT = np.ascontiguousarray(sigmas.T)
    in_maps = [
        {"muT": np.ascontiguousarray(np.roll(muT, -RB * k, axis=1)),
         "sigmaT": np.ascontiguousarray(np.roll(sgT, -RB * k, axis=1))}
        for k in range(NC)
    ]
    kwargs = {}
    if trace:
        kwargs = dict(trace=True, trace_cores=[0])
    br = run_bass_kernel_spmd(_get_nc(), in_maps, core_ids=list(range(NC)),
                              **kwargs)
    total = sum(float(r["out"].astype(np.float64).sum()) for r in br.results)
    n_classes = NB - 1
    to_mult = (n_classes - 1.0 / WEIGHT) / (n_classes - 1)
    to_add = -np.log(np.float32(to_mult))
    loss = np.float32(total / NB - to_add)
    return loss, br


def kernel(z_hat, mu_x, sigma_x, mu_p, sigma_p):
    loss, _ = run_sharded(mu_x, sigma_x, mu_p, sigma_p)
    return np.asarray(loss, np.float32)


# revision 15
# speedup vs baseline: 1.1147x; 1.1147x over previous
"""Trainium2 Bass kernel for nn_ContrastiveDistortion (symmetric pairwise-KL InfoNCE loss).

Math: with IS_SYMMETRIC=True the logdet terms cancel and
  logits_sym[a,b] = D/2 - U[a,b]/4
With w = sigma^2 + mu^2, inv = 1/sigma^2, m2i = -2*mu*inv, niv = mu*inv,
mu2 = -2*mu:
  U[a,b] = <inv_a, w_b> + <w_a, inv_b> + <m2i_a, mu_b> + <mu2_a, niv_b> + c_b + c_a
where c_x = sum_d (mu^2*inv)_x. The row term c_a cancels in log-softmax, and
c_b is computed as sum_d (w*inv)_b = c_b + D (the uniform +D also cancels), so
only FIVE K=128 matmul chunks per output tile are needed (vs 7 naive).

Each of the 8 cores gets the full [128,4096] feature-major mu/sigma,
column-ROTATED by 512*k so the program is SPMD-identical: the core's own
512-row block is always local columns 0..511 (diagonal masked there) and the
positive pairs are local columns 2048..2559.

Pipeline over eight 512-column tiles: DMA prefetch (2 ahead) -> plane prep
(1 ahead, spread over Act/DVE/Pool) -> 20 matmuls/tile -> online-softmax
consumers. PSUM holds four [128,1024] accumulators (one per 128-row m-chunk,
covering a PAIR of column tiles = all 16KB of PSUM), so the min-reduce (Pool)
and exp+accumulate (Act) each run once per 1024 columns. 16 per-(m,pair)
partial (min, sum-exp) pairs are combined with a small logsumexp tree at the
end; per-core partial row-loss sums are reduced on host.
"""

import sys
from contextlib import ExitStack

import numpy as np

sys.path.insert(0, "/opt/trn_rl_repo")

import concourse.bass as bass
import concourse.bacc as bacc_mod
import concourse.mybir as mybir
from concourse.bass_utils import run_bass_kernel_spmd
from concourse.tile import TileContext

F32 = mybir.dt.float32
F32R = mybir.dt.float32r
I32 = mybir.dt.int32
AF = mybir.ActivationFunctionType
ALU = mybir.AluOpType
AX = mybir.AxisListType

P = 128          # partitions / feature dim D
NB = 4096        # N = 2B rows
NC = 8           # cores
RB = NB // NC    # 512 rows per core
NM = RB // P     # 4 m-chunks of 128 rows
CT = 512         # column tile
NCT = NB // CT   # 8 column tiles
NPAIR = NCT // 2  # 4 column-tile pairs (one PSUM accumulator lifetime each)
NPART = NM * NPAIR  # 16 (m, pair) softmax partials
TEMPERATURE = 0.1
WEIGHT = 5.0
SCL = 1.0 / (4.0 * TEMPERATURE)  # 2.5: logit = -SCL*U + const_row
BIG = 1e30


def _build_nc():
    nc = bacc_mod.Bacc(None, target_bir_lowering=False, name="contrastive_distortion")
    muT_d = nc.declare_dram_parameter("muT", [P, NB], F32R, isOutput=False)
    sgT_d = nc.declare_dram_parameter("sigmaT", [P, NB], F32R, isOutput=False)
    out_d = nc.declare_dram_parameter("out", [P, 2 * NPART + NM], F32,
                                      isOutput=True)

    with TileContext(nc) as tc, ExitStack() as ctx:
        per = ctx.enter_context(tc.tile_pool(name="per", bufs=1))
        io = ctx.enter_context(tc.tile_pool(name="io", bufs=3))
        st = ctx.enter_context(tc.tile_pool(name="st", bufs=2))
        sc = ctx.enter_context(tc.tile_pool(name="sc", bufs=2))
        pp = ctx.enter_context(tc.tile_pool(name="pp", bufs=1, space="PSUM"))

        ones_f = per.tile([P, P], F32)
        ones = per.tile([P, P], F32R)
        ioti = per.tile([P, P], I32)
        I128 = per.tile([P, P], F32)
        # out36 packs the 16 (m,pair) softmax partials (min, sumexp) plus the
        # 4 positive-pair U values; the final logsumexp combine runs on host
        out36 = per.tile([P, 2 * NPART + NM], F32)
        pmin = out36[:, 0:NPART]
        esum = out36[:, NPART:2 * NPART]
        upos4 = out36[:, 2 * NPART:2 * NPART + NM]
        pminh = per.tile([P, 2 * NPART], F32)  # per-512-half partial mins
        biasb = per.tile([P, NPART], F32)

        nc.vector.memset(ones_f, 1.0)
        nc.vector.tensor_copy(out=ones, in_=ones_f)
        # I128[p, c] = (c - p == 0): identity for diag mask / positive extract
        nc.gpsimd.iota(ioti, pattern=[[1, P]], base=0, channel_multiplier=-1)
        nc.vector.tensor_single_scalar(out=I128, in_=ioti, scalar=0,
                                       op=ALU.is_equal)

        # persistent ct0 planes: lhsT for every matmul + rhs for ct0
        mu0 = per.tile([P, CT], F32R)
        sg0 = per.tile([P, CT], F32R)
        var0 = per.tile([P, CT], F32R)
        msq0 = per.tile([P, CT], F32R)
        inv0 = per.tile([P, CT], F32R)
        w0 = per.tile([P, CT], F32R)
        m2i0 = per.tile([P, CT], F32R)
        mu20 = per.tile([P, CT], F32R)
        niv0 = per.tile([P, CT], F32R)
        wiv0 = per.tile([P, CT], F32R)

        def dma_in(ct):
            mu_c = io.tile([P, CT], F32R, name=f"mu{ct}", tag="mu")
            sg_c = io.tile([P, CT], F32R, name=f"sg{ct}", tag="sg")
            nc.sync.dma_start(out=mu_c, in_=muT_d[:, CT * ct:CT * (ct + 1)])
            nc.sync.dma_start(out=sg_c, in_=sgT_d[:, CT * ct:CT * (ct + 1)])
            return mu_c, sg_c

        nc.sync.dma_start(out=mu0, in_=muT_d[:, 0:CT])
        nc.sync.dma_start(out=sg0, in_=sgT_d[:, 0:CT])
        iot = {1: dma_in(1)}

        # ct0 planes: latency-optimized engine spread (Act/DVE/Pool parallel)
        nc.scalar.activation(out=var0, in_=sg0, func=AF.Square)
        nc.vector.tensor_mul(msq0, mu0, mu0)
        with nc.allow_low_precision("planes feed the PE which reads fp22"):
            nc.vector.reciprocal(inv0, var0)
        nc.gpsimd.tensor_add(w0, var0, msq0)
        nc.vector.scalar_tensor_tensor(out=m2i0, in0=mu0, scalar=-2.0,
                                       in1=inv0, op0=ALU.mult, op1=ALU.mult)
        nc.vector.tensor_scalar_mul(mu20, mu0, -2.0)
        nc.gpsimd.tensor_mul(niv0, mu0, inv0)
        nc.gpsimd.tensor_mul(wiv0, w0, inv0)
        planes = {0: (mu0, w0, inv0, niv0, wiv0)}

        def mk_planes(ct, mu_c, sg_c):
            # steady state: Act squares, DVE reciprocal (+pair mins), Pool
            # the TensorTensor ops (walrus rejects TensorScalar* on Pool)
            var = st.tile([P, CT], F32R, name=f"var{ct}", tag="var")
            msq = st.tile([P, CT], F32R, name=f"msq{ct}", tag="msq")
            inv = st.tile([P, CT], F32R, name=f"inv{ct}", tag="inv")
            w = st.tile([P, CT], F32R, name=f"w{ct}", tag="w")
            niv = st.tile([P, CT], F32R, name=f"niv{ct}", tag="niv")
            wiv = st.tile([P, CT], F32R, name=f"wiv{ct}", tag="wiv")
            nc.scalar.activation(out=var, in_=sg_c, func=AF.Square)
            nc.scalar.activation(out=msq, in_=mu_c, func=AF.Square)
            with nc.allow_low_precision("planes feed the PE which reads fp22"):
                nc.vector.reciprocal(inv, var)
            nc.gpsimd.tensor_add(w, var, msq)
            nc.gpsimd.tensor_mul(niv, mu_c, inv)
            nc.gpsimd.tensor_mul(wiv, w, inv)
            return (mu_c, w, inv, niv, wiv)

        psum = {}

        def mk_mms(ct):
            mu_c, w, inv, niv, wiv = planes[ct]
            pair, half = divmod(ct, 2)
            osl = slice(CT * half, CT * (half + 1))
            for m in range(NM):
                mblk = slice(P * m, P * (m + 1))
                if half == 0:
                    psum[(pair, m)] = pp.tile([P, 2 * CT], F32,
                                              name=f"u{pair}_{m}", tag=f"ps{m}")
                u = psum[(pair, m)]
                # rhs=mu first: its DMA finished long ago, keeping the
                # group-head matmul's wait set minimal
                chunks = [(m2i0[:, mblk], mu_c), (inv0[:, mblk], w),
                          (w0[:, mblk], inv), (mu20[:, mblk], niv),
                          (ones, wiv)]
                for ci, (lt, rt) in enumerate(chunks):
                    nc.tensor.matmul(u[:, osl], lhsT=lt, rhs=rt,
                                     start=(ci == 0),
                                     stop=(ci == len(chunks) - 1))
                # per-half min right after each group so the consumer chain
                # finishes inside the next matmul window (no PE stall at
                # psum-slot reuse, which would also reset the PE pstate ramp)
                idx = NPAIR * m + pair
                if ct == 0:
                    # exclude the diagonal (always local cols 128m+p)
                    nc.vector.scalar_tensor_tensor(
                        out=u[:, mblk], in0=I128, scalar=BIG, in1=u[:, mblk],
                        op0=ALU.mult, op1=ALU.add)
                if ct == 4:
                    # positive logits live at local cols 2048+128m+p
                    s128 = sc.tile([P, P], F32, name=f"s{m}", tag="s128",
                                   bufs=1)
                    nc.vector.tensor_mul(s128, u[:, mblk], I128)
                    nc.vector.tensor_reduce(upos4[:, m:m + 1], s128,
                                            axis=AX.X, op=ALU.add)
                nc.vector.tensor_reduce(pminh[:, 2 * idx + half:2 * idx + half + 1],
                                        u[:, osl], axis=AX.X, op=ALU.min)

        def mk_cons(pair):
            for m in range(NM):
                u = psum[(pair, m)]
                idx = NPAIR * m + pair
                nc.vector.tensor_reduce(pmin[:, idx:idx + 1],
                                        pminh[:, 2 * idx:2 * idx + 2],
                                        axis=AX.X, op=ALU.min)
                nc.vector.tensor_scalar_mul(biasb[:, idx:idx + 1],
                                            pmin[:, idx:idx + 1], SCL)
                e = sc.tile([P, 2 * CT], F32, name=f"e{pair}_{m}", tag="e",
                            bufs=2)
                nc.scalar.activation(out=e, in_=u, func=AF.Exp,
                                     bias=biasb[:, idx:idx + 1], scale=-SCL,
                                     accum_out=esum[:, idx:idx + 1])

        for ct in range(NCT):
            if ct + 2 < NCT:
                iot[ct + 2] = dma_in(ct + 2)
            if ct + 1 < NCT:
                planes[ct + 1] = mk_planes(ct + 1, *iot.pop(ct + 1))
            mk_mms(ct)
            if ct % 2 == 1:
                mk_cons(ct // 2)

        # final logsumexp combine over the 4 pair-partials runs on HOST
        nc.sync.dma_start(out=out_d[:, :], in_=out36)

    return nc


_NC_CACHE = None


def _get_nc():
    global _NC_CACHE
    if _NC_CACHE is None:
        nc = _build_nc()
        nc.finalize()  # runs Bacc.compile(): wait legalization for TRN2
        _NC_CACHE = nc
    return _NC_CACHE


def run_sharded(mu_x, sigma_x, mu_p, sigma_p, trace=False):
    mus = np.concatenate([np.asarray(mu_x, np.float32),
                          np.asarray(mu_p, np.float32)], 0)
    sigmas = np.concatenate([np.asarray(sigma_x, np.float32),
                             np.asarray(sigma_p, np.float32)], 0)
    muT = np.ascontiguousarray(mus.T)
    sgT = np.ascontiguousarray(sigmas.T)
    in_maps = [
        {"muT": np.ascontiguousarray(np.roll(muT, -RB * k, axis=1)),
         "sigmaT": np.ascontiguousarray(np.roll(sgT, -RB * k, axis=1))}
        for k in range(NC)
    ]
    kwargs = {}
    if trace:
        kwargs = dict(trace=True, trace_cores=[0])
    br = run_bass_kernel_spmd(_get_nc(), in_maps, core_ids=list(range(NC)),
                              **kwargs)
    # host-side final combine: per row, LSE over the 4 (min, sumexp) pair
    # partials, then row_loss = LSE + SCL * U_pos; mean over all rows
    total = 0.0
    for r in br.results:
        arr = r["out"].astype(np.float64)        # [128, 36]
        pm = arr[:, :NPART].reshape(P, NM, NPAIR)
        es = arr[:, NPART:2 * NPART].reshape(P, NM, NPAIR)
        up = arr[:, 2 * NPART:2 * NPART + NM]    # [128, 4]
        L = -SCL * pm + np.log(es)               # partial LSEs
        M = L.max(axis=2)
        lse = M + np.log(np.exp(L - M[:, :, None]).sum(axis=2))
        total += float((lse + SCL * up).sum())
    n_classes = NB - 1
    to_mult = (n_classes - 1.0 / WEIGHT) / (n_classes - 1)
    to_add = -np.log(np.float32(to_mult))
    loss = np.float32(total / NB - to_add)
    return loss, br


def kernel(z_hat, mu_x, sigma_x, mu_p, sigma_p):
    loss, _ = run_sharded(mu_x, sigma_x, mu_p, sigma_p)
    return np.asarray(loss, np.float32)


# revision 18
# speedup vs baseline: 1.2065x; 1.0824x over previous
"""Trainium2 Bass kernel for nn_ContrastiveDistortion (symmetric pairwise-KL InfoNCE loss).

Math: with IS_SYMMETRIC=True the logdet terms cancel and
  logits_sym[a,b] = D/2 - U[a,b]/4
With w = sigma^2 + mu^2, inv = 1/sigma^2, m2i = -2*mu*inv, niv = mu*inv,
mu2 = -2*mu:
  U[a,b] = <inv_a, w_b> + <w_a, inv_b> + <m2i_a, mu_b> + <mu2_a, niv_b> + c_b + c_a
where c_x = sum_d (mu^2*inv)_x. The row term c_a cancels in log-softmax, and
c_b is computed as sum_d (w*inv)_b = c_b + D (the uniform +D also cancels), so
only FIVE K=128 matmul chunks per output tile are needed (vs 7 naive).

Each of the 8 cores gets the full [128,4096] feature-major mu/sigma,
column-ROTATED by 512*k so the program is SPMD-identical: the core's own
512-row block is always local columns 0..511 (diagonal masked there) and the
positive pairs are local columns 2048..2559.

Pipeline over eight 512-column tiles: DMA prefetch (2 ahead) -> plane prep
(1 ahead, spread over Act/DVE/Pool) -> 20 matmuls/tile -> online-softmax
consumers. PSUM holds four [128,1024] accumulators (one per 128-row m-chunk,
covering a PAIR of column tiles = all 16KB of PSUM), so the min-reduce (Pool)
and exp+accumulate (Act) each run once per 1024 columns. 16 per-(m,pair)
partial (min, sum-exp) pairs are combined with a small logsumexp tree at the
end; per-core partial row-loss sums are reduced on host.
"""

import sys
from contextlib import ExitStack

import numpy as np

sys.path.insert(0, "/opt/trn_rl_repo")

import concourse.bass as bass
import concourse.bacc as bacc_mod
import concourse.mybir as mybir
from concourse.bass_utils import run_bass_kernel_spmd
from concourse.tile import TileContext

F32 = mybir.dt.float32
F32R = mybir.dt.float32r
I32 = mybir.dt.int32
AF = mybir.ActivationFunctionType
ALU = mybir.AluOpType
AX = mybir.AxisListType

P = 128          # partitions / feature dim D
NB = 4096        # N = 2B rows
NC = 8           # cores
RB = NB // NC    # 512 rows per core
NM = RB // P     # 4 m-chunks of 128 rows
CT = 512         # column tile
NCT = NB // CT   # 8 column tiles
NPAIR = NCT // 2  # 4 column-tile pairs (one PSUM accumulator lifetime each)
NPART = NM * NPAIR  # 16 (m, pair) softmax partials
TEMPERATURE = 0.1
WEIGHT = 5.0
SCL = 1.0 / (4.0 * TEMPERATURE)  # 2.5: logit = -SCL*U + const_row
BIG = 1e30


def _build_nc():
    nc = bacc_mod.Bacc(None, target_bir_lowering=False, name="contrastive_distortion")
    muT_d = nc.declare_dram_parameter("muT", [P, NB], F32R, isOutput=False)
    sgT_d = nc.declare_dram_parameter("sigmaT", [P, NB], F32R, isOutput=False)
    out_d = nc.declare_dram_parameter("out", [P, 2 * NPART + NM], F32,
                                      isOutput=True)

    with TileContext(nc) as tc, ExitStack() as ctx:
        per = ctx.enter_context(tc.tile_pool(name="per", bufs=1))
        io = ctx.enter_context(tc.tile_pool(name="io", bufs=3))
        st = ctx.enter_context(tc.tile_pool(name="st", bufs=2))
        sc = ctx.enter_context(tc.tile_pool(name="sc", bufs=2))
        pp = ctx.enter_context(tc.tile_pool(name="pp", bufs=1, space="PSUM"))

        ones_f = per.tile([P, P], F32)
        ones = per.tile([P, P], F32R)
        ioti = per.tile([P, P], I32)
        I128 = per.tile([P, P], F32)
        # out36 packs the 16 (m,pair) softmax partials (min, sumexp) plus the
        # 4 positive-pair U values; the final logsumexp combine runs on host
        out36 = per.tile([P, 2 * NPART + NM], F32)
        pmin = out36[:, 0:NPART]
        esum = out36[:, NPART:2 * NPART]
        upos4 = out36[:, 2 * NPART:2 * NPART + NM]
        pminh = per.tile([P, 2 * NPART], F32)  # per-512-half partial mins
        biasb = per.tile([P, NPART], F32)

        nc.vector.memset(ones_f, 1.0)
        nc.vector.tensor_copy(out=ones, in_=ones_f)
        # I128[p, c] = (c - p == 0): identity for diag mask / positive extract
        nc.gpsimd.iota(ioti, pattern=[[1, P]], base=0, channel_multiplier=-1)
        nc.vector.tensor_single_scalar(out=I128, in_=ioti, scalar=0,
                                       op=ALU.is_equal)

        # persistent ct0 planes: lhsT for every matmul + rhs for ct0
        mu0 = per.tile([P, CT], F32R)
        sg0 = per.tile([P, CT], F32R)
        var0 = per.tile([P, CT], F32R)
        msq0 = per.tile([P, CT], F32R)
        inv0 = per.tile([P, CT], F32R)
        w0 = per.tile([P, CT], F32R)
        m2i0 = per.tile([P, CT], F32R)
        mu20 = per.tile([P, CT], F32R)
        niv0 = per.tile([P, CT], F32R)
        wiv0 = per.tile([P, CT], F32R)

        def dma_in(ct):
            mu_c = io.tile([P, CT], F32R, name=f"mu{ct}", tag="mu")
            sg_c = io.tile([P, CT], F32R, name=f"sg{ct}", tag="sg")
            nc.sync.dma_start(out=mu_c, in_=muT_d[:, CT * ct:CT * (ct + 1)])
            nc.sync.dma_start(out=sg_c, in_=sgT_d[:, CT * ct:CT * (ct + 1)])
            return mu_c, sg_c

        # sg0 first: the var0->inv0 chain is the prologue critical path and
        # the DMA device serializes transfers
        nc.sync.dma_start(out=sg0, in_=sgT_d[:, 0:CT])
        nc.sync.dma_start(out=mu0, in_=muT_d[:, 0:CT])
        iot = {1: dma_in(1)}

        # ct0 planes: latency-optimized engine spread (Act/DVE/Pool parallel)
        nc.scalar.activation(out=var0, in_=sg0, func=AF.Square)
        nc.scalar.activation(out=msq0, in_=mu0, func=AF.Square)
        with nc.allow_low_precision("planes feed the PE which reads fp22"):
            nc.vector.reciprocal(inv0, var0)
        nc.gpsimd.tensor_add(w0, var0, msq0)
        nc.vector.scalar_tensor_tensor(out=m2i0, in0=mu0, scalar=-2.0,
                                       in1=inv0, op0=ALU.mult, op1=ALU.mult)
        nc.vector.tensor_scalar_mul(mu20, mu0, -2.0)
        nc.vector.tensor_mul(niv0, mu0, inv0)
        H = CT // 2
        nc.vector.tensor_mul(wiv0[:, 0:H], w0[:, 0:H], inv0[:, 0:H])
        nc.gpsimd.tensor_mul(wiv0[:, H:CT], w0[:, H:CT], inv0[:, H:CT])
        planes = {0: (mu0, w0, inv0, niv0, wiv0)}

        def mk_planes(ct, mu_c, sg_c):
            # steady state: Act squares, DVE reciprocal (+pair mins), Pool
            # the TensorTensor ops (walrus rejects TensorScalar* on Pool)
            var = st.tile([P, CT], F32R, name=f"var{ct}", tag="var")
            msq = st.tile([P, CT], F32R, name=f"msq{ct}", tag="msq")
            inv = st.tile([P, CT], F32R, name=f"inv{ct}", tag="inv")
            w = st.tile([P, CT], F32R, name=f"w{ct}", tag="w")
            niv = st.tile([P, CT], F32R, name=f"niv{ct}", tag="niv")
            wiv = st.tile([P, CT], F32R, name=f"wiv{ct}", tag="wiv")
            nc.scalar.activation(out=var, in_=sg_c, func=AF.Square)
            nc.scalar.activation(out=msq, in_=mu_c, func=AF.Square)
            with nc.allow_low_precision("planes feed the PE which reads fp22"):
                nc.vector.reciprocal(inv, var)
            nc.gpsimd.tensor_add(w, var, msq)
            nc.gpsimd.tensor_mul(niv, mu_c, inv)
            nc.gpsimd.tensor_mul(wiv, w, inv)
            return (mu_c, w, inv, niv, wiv)

        psum = {}

        def mk_mms(ct):
            mu_c, w, inv, niv, wiv = planes[ct]
            pair, half = divmod(ct, 2)
            osl = slice(CT * half, CT * (half + 1))
            for m in range(NM):
                mblk = slice(P * m, P * (m + 1))
                if half == 0:
                    psum[(pair, m)] = pp.tile([P, 2 * CT], F32,
                                              name=f"u{pair}_{m}", tag=f"ps{m}")
                u = psum[(pair, m)]
                # rhs=mu first: its DMA finished long ago, keeping the
                # group-head matmul's wait set minimal
                chunks = [(m2i0[:, mblk], mu_c), (inv0[:, mblk], w),
                          (w0[:, mblk], inv), (mu20[:, mblk], niv),
                          (ones, wiv)]
                for ci, (lt, rt) in enumerate(chunks):
                    nc.tensor.matmul(u[:, osl], lhsT=lt, rhs=rt,
                                     start=(ci == 0),
                                     stop=(ci == len(chunks) - 1))
                # per-half min right after each group, and on the odd half the
                # whole consumer chain (comb -> bias -> exp) immediately — all
                # interleaved per m so the m0 chain is never head-of-line
                # blocked behind m1..m3 mins on the in-order DVE queue; the
                # exp must finish before this psum slot's next-pair reuse (PE
                # stalls there AND resets its pstate ramp)
                idx = NPAIR * m + pair
                if ct == 0:
                    # exclude the diagonal (always local cols 128m+p)
                    nc.vector.scalar_tensor_tensor(
                        out=u[:, mblk], in0=I128, scalar=BIG, in1=u[:, mblk],
                        op0=ALU.mult, op1=ALU.add)
                if ct == 4:
                    # positive logits live at local cols 2048+128m+p
                    s128 = sc.tile([P, P], F32, name=f"s{m}", tag="s128",
                                   bufs=1)
                    nc.vector.tensor_mul(s128, u[:, mblk], I128)
                    nc.vector.tensor_reduce(upos4[:, m:m + 1], s128,
                                            axis=AX.X, op=ALU.add)
                nc.vector.tensor_reduce(pminh[:, 2 * idx + half:2 * idx + half + 1],
                                        u[:, osl], axis=AX.X, op=ALU.min)
                if half == 1:
                    nc.vector.tensor_reduce(pmin[:, idx:idx + 1],
                                            pminh[:, 2 * idx:2 * idx + 2],
                                            axis=AX.X, op=ALU.min)
                    nc.vector.tensor_scalar_mul(biasb[:, idx:idx + 1],
                                                pmin[:, idx:idx + 1], SCL)
                    e = sc.tile([P, 2 * CT], F32, name=f"e{pair}_{m}",
                                tag="e", bufs=2)
                    nc.scalar.activation(out=e, in_=u, func=AF.Exp,
                                         bias=biasb[:, idx:idx + 1],
                                         scale=-SCL,
                                         accum_out=esum[:, idx:idx + 1])

        for ct in range(NCT):
            if ct + 2 < NCT:
                iot[ct + 2] = dma_in(ct + 2)
            if ct + 1 < NCT:
                planes[ct + 1] = mk_planes(ct + 1, *iot.pop(ct + 1))
            mk_mms(ct)

        # final logsumexp combine over the 4 pair-partials runs on HOST
        nc.sync.dma_start(out=out_d[:, :], in_=out36)

    return nc


_NC_CACHE = None


def _get_nc():
    global _NC_CACHE
    if _NC_CACHE is None:
        nc = _build_nc()
        nc.finalize()  # runs Bacc.compile(): wait legalization for TRN2
        _NC_CACHE = nc
    return _NC_CACHE


def run_sharded(mu_x, sigma_x, mu_p, sigma_p, trace=False):
    mus = np.concatenate([np.asarray(mu_x, np.float32),
                          np.asarray(mu_p, np.float32)], 0)
    sigmas = np.concatenate([np.asarray(sigma_x, np.float32),
                             np.asarray(sigma_p, np.float32)], 0)
    muT = np.ascontiguousarray(mus.T)
    sgT = np.ascontiguousarray(sigmas.T)
    in_maps = [
        {"muT": np.ascontiguousarray(np.roll(muT, -RB * k, axis=1)),
         "sigmaT": np.ascontiguousarray(np.roll(sgT, -RB * k, axis=1))}
        for k in range(NC)
    ]
    kwargs = {}
    if trace:
        kwargs = dict(trace=True, trace_cores=[0])
    br = run_bass_kernel_spmd(_get_nc(), in_maps, core_ids=list(range(NC)),
                              **kwargs)
    # host-side final combine: per row, LSE over the 4 (min, sumexp) pair
    # partials, then row_loss = LSE + SCL * U_pos; mean over all rows
    total = 0.0
    for r in br.results:
        arr = r["out"].astype(np.float64)        # [128, 36]
        pm = arr[:, :NPART].reshape(P, NM, NPAIR)
        es = arr[:, NPART:2 * NPART].reshape(P, NM, NPAIR)
        up = arr[:, 2 * NPART:2 * NPART + NM]    # [128, 4]
        L = -SCL * pm + np.log(es)               # partial LSEs
        M = L.max(axis=2)
        lse = M + np.log(np.exp(L - M[:, :, None]).sum(axis=2))
        total += float((lse + SCL * up).sum())
    n_classes = NB - 1
    to_mult = (n_classes - 1.0 / WEIGHT) / (n_classes - 1)
    to_add = -np.log(np.float32(to_mult))
    loss = np.float32(total / NB - to_add)
    return loss, br


def kernel(z_hat, mu_x, sigma_x, mu_p, sigma_p):
    loss, _ = run_sharded(mu_x, sigma_x, mu_p, sigma_p)
    return np.asarray(loss, np.float32)


# revision 27
# speedup vs baseline: 1.3041x; 1.0809x over previous
"""Trainium2 Bass kernel for nn_ContrastiveDistortion (symmetric pairwise-KL InfoNCE loss).

Math: with IS_SYMMETRIC=True the logdet terms cancel and
  logits_sym[a,b] = D/2 - U[a,b]/4
With w = sigma^2 + mu^2, inv = 1/sigma^2, m2i = -2*mu*inv, niv = mu*inv,
mu2 = -2*mu:
  U[a,b] = <inv_a, w_b> + <w_a, inv_b> + <m2i_a, mu_b> + <mu2_a, niv_b> + c_b + c_a
where c_x = sum_d (mu^2*inv)_x. The row term c_a cancels in log-softmax, and
c_b is computed as sum_d (w*inv)_b = c_b + D (the uniform +D also cancels), so
only FIVE K=128 matmul chunks per output tile are needed (vs 7 naive).

Each of the 8 cores gets the full [128,4096] feature-major mu/sigma,
column-ROTATED by 512*k so the program is SPMD-identical: the core's own
512-row block is always local columns 0..511 (diagonal masked there) and the
positive pairs are local columns 2048..2559.

Pipeline over eight 512-column tiles: DMA prefetch (2 ahead) -> plane prep
(1 ahead, spread over Act/DVE/Pool) -> 20 matmuls/tile -> online-softmax
consumers. PSUM holds four [128,1024] accumulators (one per 128-row m-chunk,
covering a PAIR of column tiles = all 16KB of PSUM), so the min-reduce (Pool)
and exp+accumulate (Act) each run once per 1024 columns. 16 per-(m,pair)
partial (min, sum-exp) pairs are combined with a small logsumexp tree at the
end; per-core partial row-loss sums are reduced on host.
"""

import sys
from contextlib import ExitStack

import numpy as np

sys.path.insert(0, "/opt/trn_rl_repo")

import concourse.bass as bass
import concourse.bacc as bacc_mod
import concourse.mybir as mybir
from concourse.bass_utils import run_bass_kernel_spmd
from concourse.tile import TileContext

F32 = mybir.dt.float32
F32R = mybir.dt.float32r
I32 = mybir.dt.int32
AF = mybir.ActivationFunctionType
ALU = mybir.AluOpType
AX = mybir.AxisListType

P = 128          # partitions / feature dim D
NB = 4096        # N = 2B rows
NC = 8           # cores
RB = NB // NC    # 512 rows per core
NM = RB // P     # 4 m-chunks of 128 rows
CT = 512         # column tile
NCT = NB // CT   # 8 column tiles
NPAIR = NCT // 2  # 4 column-tile pairs (one PSUM accumulator lifetime each)
NPART = NM * NPAIR  # 16 (m, pair) softmax partials
TEMPERATURE = 0.1
WEIGHT = 5.0
SCL = 1.0 / (4.0 * TEMPERATURE)  # 2.5: logit = -SCL*U + const_row
BIG = 1e30


def _build_nc():
    nc = bacc_mod.Bacc(None, target_bir_lowering=False, name="contrastive_distortion")
    muT_d = nc.declare_dram_parameter("muT", [P, NB], F32R, isOutput=False)
    sgT_d = nc.declare_dram_parameter("sigmaT", [P, NB], F32R, isOutput=False)
    out_d = nc.declare_dram_parameter("out", [P, 2 * NPART + NM], F32,
                                      isOutput=True)

    with TileContext(nc) as tc, ExitStack() as ctx:
        per = ctx.enter_context(tc.tile_pool(name="per", bufs=1))
        io = ctx.enter_context(tc.tile_pool(name="io", bufs=3))
        st = ctx.enter_context(tc.tile_pool(name="st", bufs=2))
        sc = ctx.enter_context(tc.tile_pool(name="sc", bufs=2))
        pp = ctx.enter_context(tc.tile_pool(name="pp", bufs=1, space="PSUM"))

        ones_f = per.tile([P, P], F32)
        ones = per.tile([P, P], F32R)
        ioti = per.tile([P, P], I32)
        I128 = per.tile([P, P], F32)
        # out36 packs the 16 (m,pair) softmax partials (min, sumexp) plus the
        # 4 positive-pair U values; the final logsumexp combine runs on host
        out36 = per.tile([P, 2 * NPART + NM], F32)
        pmin = out36[:, 0:NPART]
        esum = out36[:, NPART:2 * NPART]
        upos4 = out36[:, 2 * NPART:2 * NPART + NM]
        pminh = per.tile([P, 2 * NPART], F32)  # per-512-half partial mins

        # SCL is folded into every lhsT plane (and `ones`), so PSUM
        # accumulates SCL*U directly: the per-half min IS the exp bias and
        # no per-partial bias scaling op is needed
        nc.vector.memset(ones_f, SCL)
        nc.vector.tensor_copy(out=ones, in_=ones_f)
        # I128[p, c] = (c - p == 0): identity for diag mask / positive extract
        nc.gpsimd.iota(ioti, pattern=[[1, P]], base=0, channel_multiplier=-1)
        nc.vector.tensor_single_scalar(out=I128, in_=ioti, scalar=0,
                                       op=ALU.is_equal)

        # persistent ct0 planes: lhsT for every matmul + rhs for ct0
        mu0 = per.tile([P, CT], F32R)
        sg0 = per.tile([P, CT], F32R)
        var0 = per.tile([P, CT], F32R)
        msq0 = per.tile([P, CT], F32R)
        inv0 = per.tile([P, CT], F32R)
        w0 = per.tile([P, CT], F32R)
        m2i0 = per.tile([P, CT], F32R)   # -2*SCL*mu*inv (lhsT only, pre-scaled)
        mu20 = per.tile([P, CT], F32R)   # -2*SCL*mu (lhsT only, pre-scaled)
        inv0s = per.tile([P, CT], F32R)  # SCL*inv (lhsT only)
        w0s = per.tile([P, CT], F32R)    # SCL*w (lhsT only)
        niv0 = per.tile([P, CT], F32R)
        wiv0 = per.tile([P, CT], F32R)

        def dma_in(ct):
            mu_c = io.tile([P, CT], F32R, name=f"mu{ct}", tag="mu")
            sg_c = io.tile([P, CT], F32R, name=f"sg{ct}", tag="sg")
            nc.sync.dma_start(out=mu_c, in_=muT_d[:, CT * ct:CT * (ct + 1)])
            nc.sync.dma_start(out=sg_c, in_=sgT_d[:, CT * ct:CT * (ct + 1)])
            return mu_c, sg_c

        # sg0 first: the var0->inv0 chain is the prologue critical path and
        # the DMA device serializes transfers
        nc.sync.dma_start(out=sg0, in_=sgT_d[:, 0:CT])
        nc.sync.dma_start(out=mu0, in_=muT_d[:, 0:CT])
        iot = {1: dma_in(1)}

        # ct0 planes: latency-optimized engine spread (Act/DVE/Pool parallel)
        nc.scalar.activation(out=var0, in_=sg0, func=AF.Square)
        nc.scalar.activation(out=msq0, in_=mu0, func=AF.Square)
        with nc.allow_low_precision("planes feed the PE which reads fp22"):
            nc.vector.reciprocal(inv0, var0)
        nc.gpsimd.tensor_add(w0, var0, msq0)
        nc.vector.scalar_tensor_tensor(out=m2i0, in0=mu0, scalar=-2.0 * SCL,
                                       in1=inv0, op0=ALU.mult, op1=ALU.mult)
        nc.vector.tensor_scalar_mul(mu20, mu0, -2.0 * SCL)
        nc.vector.tensor_mul(niv0, mu0, inv0)
        nc.scalar.activation(out=inv0s, in_=inv0, func=AF.Copy, scale=SCL)
        nc.scalar.activation(out=w0s, in_=w0, func=AF.Copy, scale=SCL)
        H = CT // 2
        nc.vector.tensor_mul(wiv0[:, 0:H], w0[:, 0:H], inv0[:, 0:H])
        nc.gpsimd.tensor_mul(wiv0[:, H:CT], w0[:, H:CT], inv0[:, H:CT])
        planes = {0: (mu0, w0, inv0, niv0, wiv0)}

        def mk_planes(ct, mu_c, sg_c):
            # steady state: Act squares, DVE reciprocal (+pair mins), Pool
            # the TensorTensor ops (walrus rejects TensorScalar* on Pool)
            var = st.tile([P, CT], F32R, name=f"var{ct}", tag="var")
            msq = st.tile([P, CT], F32R, name=f"msq{ct}", tag="msq")
            inv = st.tile([P, CT], F32R, name=f"inv{ct}", tag="inv")
            w = st.tile([P, CT], F32R, name=f"w{ct}", tag="w")
            niv = st.tile([P, CT], F32R, name=f"niv{ct}", tag="niv")
            wiv = st.tile([P, CT], F32R, name=f"wiv{ct}", tag="wiv")
            nc.scalar.activation(out=var, in_=sg_c, func=AF.Square)
            nc.scalar.activation(out=msq, in_=mu_c, func=AF.Square)
            # reciprocal in halves: the greedy per-engine list scheduler fills
            # any DVE dep-gap in the min->comb->exp chain with whatever is
            # ready; two 297ns fills hurt far less than one 594ns one
            with nc.allow_low_precision("planes feed the PE which reads fp22"):
                nc.vector.reciprocal(inv[:, 0:CT // 2], var[:, 0:CT // 2])
                nc.vector.reciprocal(inv[:, CT // 2:CT], var[:, CT // 2:CT])
            nc.gpsimd.tensor_add(w, var, msq)
            nc.gpsimd.tensor_mul(niv, mu_c, inv)
            nc.gpsimd.tensor_mul(wiv, w, inv)
            return (mu_c, w, inv, niv, wiv)

        psum = {}

        def mk_mms(ct):
            mu_c, w, inv, niv, wiv = planes[ct]
            pair, half = divmod(ct, 2)
            osl = slice(CT * half, CT * (half + 1))
            for m in range(NM):
                mblk = slice(P * m, P * (m + 1))
                if half == 0:
                    psum[(pair, m)] = pp.tile([P, 2 * CT], F32,
                                              name=f"u{pair}_{m}", tag=f"ps{m}")
                u = psum[(pair, m)]
                # rhs=mu first: its DMA finished long ago, keeping the
                # group-head matmul's wait set minimal; w0s (the last-ready
                # prologue plane) goes last
                chunks = [(m2i0[:, mblk], mu_c), (inv0s[:, mblk], w),
                          (mu20[:, mblk], niv), (ones, wiv),
                          (w0s[:, mblk], inv)]
                for ci, (lt, rt) in enumerate(chunks):
                    nc.tensor.matmul(u[:, osl], lhsT=lt, rhs=rt,
                                     start=(ci == 0),
                                     stop=(ci == len(chunks) - 1))
                # per-half min right after each group, and on the odd half the
                # whole consumer chain (comb -> bias -> exp) immediately — all
                # interleaved per m so the m0 chain is never head-of-line
                # blocked behind m1..m3 mins on the in-order DVE queue; the
                # exp must finish before this psum slot's next-pair reuse (PE
                # stalls there AND resets its pstate ramp)
                idx = NPAIR * m + pair
                if ct == 0:
                    # exclude the diagonal (always local cols 128m+p)
                    nc.vector.scalar_tensor_tensor(
                        out=u[:, mblk], in0=I128, scalar=BIG, in1=u[:, mblk],
                        op0=ALU.mult, op1=ALU.add)
                if ct == 4:
                    # positive logits live at local cols 2048+128m+p
                    s128 = sc.tile([P, P], F32, name=f"s{m}", tag="s128",
                                   bufs=1)
                    nc.vector.tensor_mul(s128, u[:, mblk], I128)
                    nc.vector.tensor_reduce(upos4[:, m:m + 1], s128,
                                            axis=AX.X, op=ALU.add)
                nc.vector.tensor_reduce(pminh[:, 2 * idx + half:2 * idx + half + 1],
                                        u[:, osl], axis=AX.X, op=ALU.min)
                if half == 1:
                    nc.vector.tensor_reduce(pmin[:, idx:idx + 1],
                                            pminh[:, 2 * idx:2 * idx + 2],
                                            axis=AX.X, op=ALU.min)
                    e = sc.tile([P, 2 * CT], F32, name=f"e{pair}_{m}",
                                tag="e", bufs=2)
                    nc.scalar.activation(out=e, in_=u, func=AF.Exp,
                                         bias=pmin[:, idx:idx + 1],
                                         scale=-1.0,
                                         accum_out=esum[:, idx:idx + 1])

        # planes(ct+1) AFTER mms(ct): the tile scheduler uses emission order
        # as engine priority, and the consumer chain (min->comb->bias->exp)
        # must win DVE/Act ties over next-tile plane prep or the psum-reuse
        # matmul head stalls
        for ct in range(NCT):
            if ct + 2 < NCT:
                iot[ct + 2] = dma_in(ct + 2)
            mk_mms(ct)
            if ct + 1 < NCT:
                planes[ct + 1] = mk_planes(ct + 1, *iot.pop(ct + 1))

        # final logsumexp combine over the 4 pair-partials runs on HOST
        nc.sync.dma_start(out=out_d[:, :], in_=out36)

    return nc


_NC_CACHE = None


def _get_nc():
    global _NC_CACHE
    if _NC_CACHE is None:
        nc = _build_nc()
        nc.finalize()  # runs Bacc.compile(): wait legalization for TRN2
        _NC_CACHE = nc
    return _NC_CACHE


def run_sharded(mu_x, sigma_x, mu_p, sigma_p, trace=False):
    mus = np.concatenate([np.asarray(mu_x, np.float32),
                          np.asarray(mu_p, np.float32)], 0)
    sigmas = np.concatenate([np.asarray(sigma_x, np.float32),
                             np.asarray(sigma_p, np.float32)], 0)
    muT = np.ascontiguousarray(mus.T)
    sgT = np.ascontiguousarray(sigmas.T)
    in_maps = [
        {"muT": np.ascontiguousarray(np.roll(muT, -RB * k, axis=1)),
         "sigmaT": np.ascontiguousarray(np.roll(sgT, -RB * k, axis=1))}
        for k in range(NC)
    ]
    kwargs = {}
    if trace:
        kwargs = dict(trace=True, trace_cores=[0])
    br = run_bass_kernel_spmd(_get_nc(), in_maps, core_ids=list(range(NC)),
                              **kwargs)
    # host-side final combine: per row, LSE over the 4 (min, sumexp) pair
    # partials, then row_loss = LSE + SCL * U_pos; mean over all rows
    total = 0.0
    for r in br.results:
        arr = r["out"].astype(np.float64)        # [128, 36]
        # device accumulates SCL*U, so pm and up are already SCL-scaled
        pm = arr[:, :NPART].reshape(P, NM, NPAIR)
        es = arr[:, NPART:2 * NPART].reshape(P, NM, NPAIR)
        up = arr[:, 2 * NPART:2 * NPART + NM]    # [128, 4]
        L = -pm + np.log(es)                     # partial LSEs
        M = L.max(axis=2)
        lse = M + np.log(np.exp(L - M[:, :, None]).sum(axis=2))
        total += float((lse + up).sum())
    n_classes = NB - 1
    to_mult = (n_classes - 1.0 / WEIGHT) / (n_classes - 1)
    to_add = -np.log(np.float32(to_mult))
    loss = np.float32(total / NB - to_add)
    return loss, br


def kernel(z_hat, mu_x, sigma_x, mu_p, sigma_p):
    loss, _ = run_sharded(mu_x, sigma_x, mu_p, sigma_p)
    return np.asarray(loss, np.float32)
